# revision 27
# baseline (speedup 1.0000x reference)
"""Cross-attention kernel for 8 TRN2 NeuronCores.

Reference computation (per batch b, c=1024 tokens, dim=1024):
    q = xf @ Wq.T ; k,v = cf @ Wkv.T split
    out = softmax(q @ k.T / 32) @ v

Key algebraic fold: scores S = q k^T = xf (Wq^T Wk) cf^T.  The product
M = Wq^T Wk is a weight-only matrix, folded on the host once.  On
device this removes the whole k-projection phase (one of five matmul
phases), and the cT tiles already resident for the v-projection serve
directly as the stationary operand of the score matmul.

Sharding: data-parallel over batch (16 batches -> 2 per core), SPMD on 8
cores, no collectives.  All activations enter the device pre-transposed
(host-side) so every matmul has its contraction dim on SBUF partitions:

    uT[e,i] = M.T  @ xT          (lhsT=M[d,e],   rhs=xT[d,i]) ; u = x M
    v[j,o]  = cT.T @ WvT         (lhsT=cT[d,j],  rhs=WvT[d,o])
    ST[j,i] = c    @ uT          (lhsT=cT[e,j],  rhs=uT[e,i])  (scores^T)
    ET      = exp(ST/32)         (ACT, scale fused; no max-subtraction --
                                  logits are ~N(0,1), exp is fp32-safe)
    out'[i,o] = ET.T @ v         (lhsT=ET[j,i], rhs=v[j,o])
    l[i]      = ET.T @ ones      (same stationary weights as out')
    out[i,o]  = out' * (1/l)     (DVE per-partition scale on PSUM->SBUF copy)

The ST (transposed-scores) formulation means the softmax matrix is never
transposed on device, and l (the softmax denominator) rides on the PE as
N=1 matmuls sharing the out' stationary tiles.
"""

import os
import sys

import numpy as np


def _ensure_paths():
    for p in ("/opt/trn_rl_repo", "/root/.axon_site/_ro/trn_rl_repo"):
        if os.path.isdir(p) and p not in sys.path:
            sys.path.append(p)


try:
    import concourse.bass  # noqa: F401
except ImportError:
    _ensure_paths()

import concourse.bass as bass  # noqa: E402
import concourse.tile as tile  # noqa: E402
from concourse import bacc, mybir  # noqa: E402
from concourse import bass_utils  # noqa: E402

B, C, HH, WW = 16, 1024, 32, 32
D = HH * WW  # 1024
NCORES = 8
BPC = B // NCORES  # 2 batches per core
P = 128
KS = D // P  # 8 contraction subtiles
NT = C // P  # 8 row tiles
NH = 512  # matmul moving free dim (one PSUM bank)
SCALE = float(D) ** -0.5

CDT = mybir.dt.float16  # on-device compute dtype
NPDT = np.float16

F32 = mybir.dt.float32

WARMUP_MMS = int(os.environ.get("KERNEL_WARMUP_MMS", "14"))


def _emit(tc, xT, cT, m, wv, out):
    nc = tc.nc
    from contextlib import ExitStack

    ctx = ExitStack()
    with ctx:
        wpool = ctx.enter_context(tc.tile_pool(name="weights", bufs=1))
        iopool = ctx.enter_context(tc.tile_pool(name="io", bufs=2))
        actpool = ctx.enter_context(tc.tile_pool(name="acts", bufs=1))
        outpool = ctx.enter_context(tc.tile_pool(name="outs", bufs=3))
        smpool = ctx.enter_context(tc.tile_pool(name="small", bufs=2))
        psum = ctx.enter_context(tc.tile_pool(name="psum", bufs=6, space="PSUM"))
        psuml = ctx.enter_context(tc.tile_pool(name="psuml", bufs=2, space="PSUM"))

        # Pre-warm the PE during the startup DMA window: HAM un-throttles
        # (1.2 -> 2.4 GHz) only after ~3.4us of sustained PE activity, so a
        # burst of throwaway matmuls here means the real stream starts warm.
        # A single accumulation group keeps them back-to-back (no PSUM WAW
        # semaphore between members); N=512 keeps the per-matmul semaphore
        # overhead small relative to the work.
        if WARMUP_MMS:
            warm_in = wpool.tile([P, NH], CDT, tag="warm", name="warm_in")
            nc.vector.memset(warm_in[:], 0.0)
            warm_ps = psum.tile([P, NH], F32, tag="mm", name="warm_ps")
            for i in range(WARMUP_MMS):
                nc.tensor.matmul(
                    warm_ps[:],
                    lhsT=warm_in[:, 0:P],
                    rhs=warm_in[:],
                    start=(i == 0),
                    stop=(i == WARMUP_MMS - 1),
                )

        # Weights resident for the whole kernel; inputs for both batches
        # prefetched up front.  M is host-relaid as [ot, p, ksd, c] so each
        # e-block ot is one contiguous-per-partition 256KB DMA.  Issue order
        # follows PE consumption: M e-block 0 + x i-half 0 (first phase-A
        # group), remaining M e-blocks, x i-half 1, then batch-0 c (phase C
        # reuses it as the score stationary), then wv (phase B), then the
        # batch-1 inputs.
        m_sb = wpool.tile([P, KS, KS, P], CDT, tag="m", name="m_sb")
        wv_sb = wpool.tile([P, KS, D], CDT, tag="wv", name="wv_sb")
        x_sbs = [
            iopool.tile([P, KS, C], CDT, tag="x", name="x_sb") for _ in range(BPC)
        ]
        c_sbs = [
            iopool.tile([P, KS, C], CDT, tag="c", name="c_sb") for _ in range(BPC)
        ]
        # The 16 DMA engines round-robin packets across ALL in-flight
        # transfers, so eagerly issuing everything would starve the
        # startup-critical first group (m e-block 0 + x i-half 0, 1.25MB).
        # Those two go out alone; everything else is chained behind the
        # x i-half-0 completion: a 1-element DVE copy out of that region
        # (RAW on the DMA) followed by 1-element token memsets into each
        # later tile (WAW: its DMA then waits for the token).
        nc.sync.dma_start(m_sb[:, 0], m[0])
        nc.sync.dma_start(x_sbs[0][:, :, 0:NH], xT[0, :, :, 0:NH])

        tok = smpool.tile([1, 1], CDT, tag="tok", name="tok")
        nc.vector.tensor_copy(tok[:], x_sbs[0][0:1, 0, 0:1])

        for ot in range(1, KS):
            nc.vector.memset(m_sb[0:1, ot, 0, 0:1], 0.0)
            nc.sync.dma_start(m_sb[:, ot], m[ot])
        nc.vector.memset(x_sbs[0][0:1, 0, NH : NH + 1], 0.0)
        nc.sync.dma_start(x_sbs[0][:, :, NH:C], xT[0, :, :, NH:C])

        # Later tensors gate on phase-A progress (DVE queue pacing).
        def _gated_dma(dst_tile, src_ap):
            def fire():
                nc.vector.memset(dst_tile[0:1, 0, 0:1], 0.0)
                nc.sync.dma_start(dst_tile[:], src_ap)

            return fire

        dma_hooks = {
            (0, 0, 1): _gated_dma(c_sbs[0], cT[0]),
            (0, 0, 5): _gated_dma(wv_sb, wv[:]),
            (0, 1, 1): _gated_dma(x_sbs[1], xT[1]),
            (0, 1, 5): _gated_dma(c_sbs[1], cT[1]),
        }

        ones = wpool.tile([P, 1], CDT, tag="ones", name="ones")
        nc.vector.memset(ones[:], 1.0)

        for n in range(BPC):
            x_sb = x_sbs[n]
            c_sb = c_sbs[n]

            # ---- phase A: uT[e,i] = M.T @ xT ----
            # ih is the outer loop so the very first matmul group only needs
            # M's first e-block + the first i-half of x (the DMA stream above
            # lands those bytes first), shaving the startup stall.
            uT_sb = actpool.tile([P, KS, C], CDT, tag="uT", name="uT_sb")
            for ih in range(2):
                for ot in range(KS):
                    ps = psum.tile([P, NH], F32, tag="mm", name="ps_mm")
                    for ks in range(KS):
                        nc.tensor.matmul(
                            ps[:],
                            lhsT=m_sb[:, ot, ks, :],
                            rhs=x_sb[:, ks, ih * NH : (ih + 1) * NH],
                            start=(ks == 0),
                            stop=(ks == KS - 1),
                        )
                    nc.vector.tensor_copy(
                        uT_sb[:, ot, ih * NH : (ih + 1) * NH], ps[:]
                    )
                    hook = dma_hooks.pop((n, ih, ot), None)
                    if hook is not None:
                        hook()

            # ---- phase C: ST[j,i] = c @ uT -> ET = exp(ST/32) ----
            # cT doubles as the stationary operand: ST[j,i] = sum_e c[j,e] u[i,e].
            # Runs before phase B so B's matmuls hide the tail exp latency
            # ahead of phase D.
            eT_sb = actpool.tile([P, KS, C], CDT, tag="eT", name="eT_sb")
            for jt in range(NT):
                ps = [psum.tile([P, NH], F32, tag="mm", name="ps_mm") for _ in range(2)]
                for os_ in range(KS):
                    for ih in range(2):
                        nc.tensor.matmul(
                            ps[ih][:],
                            lhsT=c_sb[:, os_, jt * P : (jt + 1) * P],
                            rhs=uT_sb[:, os_, ih * NH : (ih + 1) * NH],
                            start=(os_ == 0),
                            stop=(os_ == KS - 1),
                        )
                for ih in range(2):
                    nc.scalar.activation(
                        eT_sb[:, jt, ih * NH : (ih + 1) * NH],
                        ps[ih][:],
                        mybir.ActivationFunctionType.Exp,
                        scale=SCALE,
                    )

            # ---- phase B: v[j,o] = cT.T @ WvT ----
            v_sb = actpool.tile([P, KS, D], CDT, tag="v", name="v_sb")
            for jt in range(NT):
                ps = [psum.tile([P, NH], F32, tag="mm", name="ps_mm") for _ in range(2)]
                for ks in range(KS):
                    for oh in range(2):
                        nc.tensor.matmul(
                            ps[oh][:],
                            lhsT=c_sb[:, ks, jt * P : (jt + 1) * P],
                            rhs=wv_sb[:, ks, oh * NH : (oh + 1) * NH],
                            start=(ks == 0),
                            stop=(ks == KS - 1),
                        )
                for oh in range(2):
                    nc.vector.tensor_copy(
                        v_sb[:, jt, oh * NH : (oh + 1) * NH], ps[oh][:]
                    )

            # ---- phase D: out'[i,o] = ET.T @ v ; l = ET.T @ ones ; scale ----
            for it in range(NT):
                o_sb = outpool.tile([P, D], CDT, tag="o", name="o_sb")
                ps = [psum.tile([P, NH], F32, tag="mm", name="ps_mm") for _ in range(2)]
                psl = psuml.tile([P, 1], F32, tag="l", name="ps_l")
                for js in range(NT):
                    lhsT = eT_sb[:, js, it * P : (it + 1) * P]
                    # l first: on the last js this lets the reciprocal start
                    # while the final out' matmuls still stream.
                    nc.tensor.matmul(
                        psl[:],
                        lhsT=lhsT,
                        rhs=ones[:, 0:1],
                        start=(js == 0),
                        stop=(js == NT - 1),
                    )
                    for oh in range(2):
                        nc.tensor.matmul(
                            ps[oh][:],
                            lhsT=lhsT,
                            rhs=v_sb[:, js, oh * NH : (oh + 1) * NH],
                            start=(js == 0),
                            stop=(js == NT - 1),
                        )
                r_it = smpool.tile([P, 1], F32, tag="r", name="r_it")
                nc.vector.reciprocal(r_it[:], psl[:])
                # fp16 out: halves the tail DMA; host upcasts.
                for oh in range(2):
                    nc.vector.tensor_scalar_mul(
                        o_sb[:, oh * NH : (oh + 1) * NH], ps[oh][:], r_it[:]
                    )
                    nc.sync.dma_start(
                        out[n, it, :, oh * NH : (oh + 1) * NH],
                        o_sb[:, oh * NH : (oh + 1) * NH],
                    )


_NC_CACHE = {}


def _build():
    if "nc" in _NC_CACHE:
        return _NC_CACHE["nc"]
    nc = bacc.Bacc("TRN2", target_bir_lowering=False, debug=False)
    xT = nc.dram_tensor("xT", [BPC, P, KS, C], CDT, kind="ExternalInput").ap()
    cT = nc.dram_tensor("cT", [BPC, P, KS, C], CDT, kind="ExternalInput").ap()
    m = nc.dram_tensor("m", [KS, P, KS, P], CDT, kind="ExternalInput").ap()
    wv = nc.dram_tensor("wv", [P, KS, D], CDT, kind="ExternalInput").ap()
    out = nc.dram_tensor("out", [BPC, NT, P, D], CDT, kind="ExternalOutput").ap()
    with tile.TileContext(nc) as tc:
        _emit(tc, xT, cT, m, wv, out)
    nc.compile()
    _NC_CACHE["nc"] = nc
    return nc


def kernel(**inputs) -> np.ndarray:
    x = np.asarray(inputs["x"], dtype=np.float32).reshape(B, C, D)
    cond = np.asarray(inputs["cond_img"], dtype=np.float32).reshape(B, C, D)
    Wq = np.asarray(inputs["Wq"], dtype=np.float32)
    Wkv = np.asarray(inputs["Wkv"], dtype=np.float32)

    # Pre-transpose on host so the contraction dim lands on partitions, and
    # fold the k-projection into the q-projection: M = Wq^T Wk, so that
    # scores = x M c^T needs no on-device k.
    xT = np.ascontiguousarray(x.transpose(0, 2, 1)).astype(NPDT)  # (B, D, C)
    cT = np.ascontiguousarray(cond.transpose(0, 2, 1)).astype(NPDT)
    M = (Wq.T @ Wkv[:D]).astype(NPDT)  # (D_in_x, D_in_c)
    wvT = np.ascontiguousarray(Wkv[D:].T).astype(NPDT)

    # Partition-major layouts so whole-tile DMAs are contiguous per
    # partition: xT[n, p, ks, i] = x[n, i, ks*P+p], one DMA per tensor.
    xT = np.ascontiguousarray(
        xT.reshape(NCORES, BPC, KS, P, C).transpose(0, 1, 3, 2, 4)
    )
    cT = np.ascontiguousarray(
        cT.reshape(NCORES, BPC, KS, P, C).transpose(0, 1, 3, 2, 4)
    )
    # m[ot, p, ksd, c] = M[ksd*P + p, ot*P + c]: each e-block ot is one
    # contiguous-per-partition DMA.
    m = np.ascontiguousarray(M.reshape(KS, P, KS, P).transpose(2, 1, 0, 3))
    wv = np.ascontiguousarray(wvT.reshape(KS, P, D).transpose(1, 0, 2))

    in_maps = [
        {"xT": xT[i], "cT": cT[i], "m": m, "wv": wv}
        for i in range(NCORES)
    ]

    nc = _build()
    trace = bool(os.environ.get("KERNEL_TRACE"))
    res = bass_utils.run_bass_kernel_spmd(
        nc, in_maps, core_ids=list(range(NCORES)), trace=trace
    )
    if trace:
        _NC_CACHE["last_result"] = res

    outs = np.stack([np.asarray(res.results[i]["out"]) for i in range(NCORES)])
    return outs.reshape(B, C, HH, WW).astype(np.float32)
